# revision 9
# baseline (speedup 1.0000x reference)
"""Trainium2 Bass kernel for nn_AttentionCell (sparse local attention, W=16).

Contract: kernel(**inputs) takes the FULL inputs
    inputs: [8, 1024, 512] f32, M/C/V: [512, 512] f32
and returns the FULL output [8, 1024, 1024] f32
    out = concat([inputs, local_attention(inputs)], axis=-1)

Sharding: data-parallel over batch - one batch element per NeuronCore (8
cores). Host-side prep: M/C are fused into G = M @ C.T so that
    logits = (x @ M) @ (x @ C).T = (x @ G) @ x.T
(no K projection on device); x, G, V are cast to bf16 on the host; the
device computes ONLY the attention half (bf16 out) and the host
concatenates [x_f32, answer_f32].  Device HBM traffic is 1MB x in +
1MB G/V in + 1MB answer out (vs 8MB for the all-f32 full-output kernel).

Per-core device algorithm (x: [1024, 512] bf16), all matmuls bf16
(1 cyc/row on PE at any free size, 2x faster transposes vs f32):
  1. xT stored zero-padded by PAD=16 cols left and 96 right so any
     128-wide window is a contiguous slice (col j = x row j-16),
     built from eight 128-row chunk transposes.
  2. Q'T = G.T @ xT (two 512-wide spans).
  3. Attention runs on 112-query chunks (9x112 + 1x16): queries
     [112k, 112k+111] attend keys [112k-16, 112k+111] - exactly 128
     rows, so V' for chunk k is ONE 128-row window of x @ Vw computed
     straight off xT (Vn[p, k] = V' row 112k-16+p), and S @ V is ONE
     K=128 matmul with no partition-misaligned tail.
  4. Banded logits [112, 128] (4 accumulating matmuls); softmax with an
     additive -1e9 band mask built on-device via affine_select (valid
     w in [q+1, q+16]; out-of-sequence keys are zero columns of xT so
     their logits are exactly 0, matching the reference zero-padding);
     scores transposed on PE, normalization folded into the PSUM->SBUF
     scale copy, output cast to bf16.

DMA plan: the two HWDGE rings are used in parallel at startup
(sync: x chunks 0-3 in two paired loads + answer stores; scalar:
G halves, x chunks 4-7, V) so sequencer config time (~0.6us per DMA)
never serializes the x stream behind the weights.  PE warm-up junk
matmuls (no data deps) open the clock gate during the load latency.
"""

import os
import sys

import numpy as np

for _p in ("/opt/trn_rl_repo", "/opt/pypackages"):
    if os.path.isdir(_p) and _p not in sys.path:
        sys.path.append(_p)

import ml_dtypes

import concourse.bacc as bacc
import concourse.tile as tile
from concourse import mybir
from concourse.bass_utils import run_bass_kernel_spmd

f32 = mybir.dt.float32
bf16 = mybir.dt.bfloat16

B = 8
T = 1024
D = 512
PAD = 16           # left halo: 15 in-window keys + 1 masked (alignment)
QCH = 112          # attention query-chunk size (key span = QCH+16 = 128)
NQC = 10           # 9 chunks of 112 + final chunk of 16
RPAD = 96          # right zero pad so chunk 9's 128-wide reads stay in range
XTW = PAD + T + RPAD
NCH = T // 128     # 128-row transpose/load chunks
NDC = D // 128     # feature chunks
MASKVAL = -1.0e9

_cache: dict = {}


def _ts(i, n=128):
    return slice(i * n, (i + 1) * n)


def _qn(k):
    return QCH if k < NQC - 1 else T - QCH * (NQC - 1)


def _emit(tc, nc, xd, Gd, Vd, outd):
    AF = mybir.ActivationFunctionType
    OP = mybir.AluOpType
    from contextlib import ExitStack

    stack = ExitStack()
    constp = stack.enter_context(tc.tile_pool(name="const", bufs=1))
    bigp = stack.enter_context(tc.tile_pool(name="big", bufs=1))
    smp = stack.enter_context(tc.tile_pool(name="sm", bufs=4))
    pTp = stack.enter_context(tc.tile_pool(name="pT", bufs=2, space="PSUM"))
    pQVp = stack.enter_context(tc.tile_pool(name="pQV", bufs=3, space="PSUM"))
    pLp = stack.enter_context(tc.tile_pool(name="pL", bufs=3, space="PSUM"))

    # --- constants (generated on device: no DMA) ---
    identity = constp.tile([128, 128], bf16)
    nc.gpsimd.memset(identity[:], 0.0)
    nc.gpsimd.affine_select(
        out=identity[:], in_=identity[:], compare_op=OP.not_equal,
        fill=1.0, base=0, pattern=[[-1, 128]], channel_multiplier=1,
    )
    # band[q, w] = 0 where q+1 <= w <= q+16 else -1e9
    band = constp.tile([128, 128], f32)
    nc.gpsimd.memset(band[:], 0.0)
    nc.gpsimd.affine_select(
        out=band[:], in_=band[:], compare_op=OP.is_ge,
        fill=MASKVAL, base=-1, pattern=[[1, 128]], channel_multiplier=-1,
    )
    nc.gpsimd.affine_select(
        out=band[:], in_=band[:], compare_op=OP.is_ge,
        fill=MASKVAL, base=PAD, pattern=[[-1, 128]], channel_multiplier=1,
    )
    zt = constp.tile([128, 128], bf16)
    nc.vector.memset(zt[:], 0.0)

    # --- weights ---
    Gw = constp.tile([128, NDC, D], bf16)
    Vws = constp.tile([128, NDC, D], bf16)

    # --- persistent activations ---
    xin = bigp.tile([128, NCH, D], bf16)     # x chunks, natural layout
    xTp = bigp.tile([128, NDC, XTW], bf16)   # x.T, zero-padded both sides
    nc.vector.memset(xTp[:, :, 0:PAD], 0.0)
    nc.vector.memset(xTp[:, :, PAD + T : XTW], 0.0)
    QT = bigp.tile([128, NDC, T], bf16)      # (x @ G).T
    Vn = bigp.tile([128, NQC, D], bf16)      # Vn[p, k] = (x@Vw) row 112k-16+p

    # --- loads: both HWDGE rings in parallel ---
    xdr = xd.rearrange("(c p) d -> p c d", p=128)
    Gdr = Gd.rearrange("(c p) n -> p c n", p=128)
    nc.sync.dma_start(xin[:, 0:2, :], xdr[:, 0:2, :])
    nc.sync.dma_start(xin[:, 2:4, :], xdr[:, 2:4, :])
    nc.scalar.dma_start(Gw[:, :, 0:256], Gdr[:, :, 0:256])
    nc.scalar.dma_start(Gw[:, :, 256:512], Gdr[:, :, 256:512])
    nc.scalar.dma_start(xin[:, 4:8, :], xdr[:, 4:8, :])
    nc.scalar.dma_start(Vws[:], Vd.rearrange("(c p) n -> p c n", p=128))

    # --- PE warm-up: junk matmuls (no data deps) open the HAM clock-gate
    # (0.65 -> 2.4 GHz) while the first x chunks load; sized to end right
    # when the first x load's semaphore fires (~2.9us after main) since
    # any PE idle gap resets the clock ramp ---
    zwide = constp.tile([128, 512], bf16)
    nc.vector.memset(zwide[:], 0.0)
    pwarm = pQVp.tile([128, 512], f32, name="pwarm", tag="pq")
    NWARM = 6
    for w in range(NWARM):
        nc.tensor.matmul(
            pwarm[:], zt[:], zwide[:], start=(w == 0), stop=(w == NWARM - 1)
        )

    # --- per-128-row-chunk transpose ---
    def load_transpose(i):
        pst = pTp.tile([128, NDC, 128], bf16, name=f"pt{i}", tag="pt")
        for dc in range(NDC):
            nc.tensor.transpose(pst[:, dc, :], xin[:, i, _ts(dc)], identity[:])
        nc.vector.tensor_copy(xTp[:, :, PAD + 128 * i : PAD + 128 * (i + 1)], pst[:])

    # --- Q' projection for one 512-wide t-span ---
    def qproj(s):
        for m in range(NDC):
            pq = pQVp.tile([128, 512], f32, name=f"pq{s}_{m}", tag="pq")
            for dc in range(NDC):
                nc.tensor.matmul(
                    pq[:],
                    Gw[:, dc, _ts(m)],
                    xTp[:, dc, PAD + 512 * s : PAD + 512 * (s + 1)],
                    start=(dc == 0),
                    stop=(dc == NDC - 1),
                )
            nc.scalar.copy(QT[:, m, _ts(s, 512)], pq[:])

    # --- V' window projection for one 112-query chunk: rows 112k-16.. ---
    def vproj(k):
        pv = pQVp.tile([128, 512], f32, name=f"pv{k}", tag="pq")
        for dc in range(NDC):
            nc.tensor.matmul(
                pv[:],
                xTp[:, dc, QCH * k : QCH * k + 128],
                Vws[:, dc, :],
                start=(dc == 0),
                stop=(dc == NDC - 1),
            )
        # k=7,9 go to DVE so the ACT queue stays clear for the final exps
        if k % 2 == 0 or k >= 7:
            nc.vector.tensor_copy(Vn[:, k, :], pv[:])
        else:
            nc.scalar.copy(Vn[:, k, :], pv[:])

    # --- banded logits for one 112-query chunk ---
    pltiles = {}

    def logits(k):
        q = _qn(k)
        pl = pLp.tile([128, 128], f32, name=f"pl{k}", tag="pl")
        for dc in range(NDC):
            nc.tensor.matmul(
                pl[0:q, :],
                QT[:, dc, QCH * k : QCH * k + q],
                xTp[:, dc, QCH * k : QCH * k + 128],
                start=(dc == 0),
                stop=(dc == NDC - 1),
            )
        pltiles[k] = pl

    # --- softmax + score transpose (PE part emitted separately from SV
    # so the DVE psum->sbuf copy of the scores never stalls the PE) ---
    sttiles = {}
    rctiles = {}

    def scoreT(k):
        q = _qn(k)
        pl = pltiles.pop(k)
        Lm = smp.tile([128, 128], f32, name=f"lm{k}", tag="lm")
        nc.vector.tensor_add(Lm[0:q, :], pl[0:q, :], band[0:q, :])
        negm = smp.tile([128, 1], f32, name=f"nm{k}", tag="nm")
        nc.vector.reduce_max(
            negm[0:q, :], Lm[0:q, :], axis=mybir.AxisListType.X, negate=True
        )
        P = smp.tile([128, 128], bf16, name=f"pp{k}", tag="pp")
        rowsum = smp.tile([128, 1], f32, name=f"rs{k}", tag="rs")
        nc.scalar.activation(
            P[0:q, :], Lm[0:q, :], AF.Exp, bias=negm[0:q, :], accum_out=rowsum[0:q, :]
        )
        recip = smp.tile([128, 1], f32, name=f"rc{k}", tag="rc")
        nc.vector.reciprocal(recip[0:q, :], rowsum[0:q, :])
        rctiles[k] = recip
        pst = pTp.tile([128, QCH], bf16, name=f"ps{k}", tag="pt")
        nc.tensor.transpose(pst[:, 0:q], P[0:q, :], identity[0:q, 0:q])
        st = smp.tile([128, QCH], bf16, name=f"st{k}", tag="st")
        nc.vector.tensor_copy(st[:, 0:q], pst[:, 0:q])
        sttiles[k] = st

    def sv_store(k):
        q = _qn(k)
        st = sttiles.pop(k)
        recip = rctiles.pop(k)
        pa = pQVp.tile([128, 512], f32, name=f"pa{k}", tag="pq")
        nc.tensor.matmul(pa[0:q, :], st[:, 0:q], Vn[:, k, :], start=True, stop=True)
        ans = smp.tile([128, 512], bf16, name=f"ans{k}", tag="ans")
        if k % 2 == 0:
            nc.scalar.mul(ans[0:q, :], pa[0:q, :], recip[0:q, :])
        else:
            nc.vector.tensor_scalar_mul(ans[0:q, :], pa[0:q, :], recip[0:q, :])
        # the last chunk's store goes on the otherwise-idle scalar ring so
        # its DMA config runs in parallel with the previous store's
        eng = nc.scalar if k == NQC - 1 else nc.sync
        eng.dma_start(outd[QCH * k : QCH * k + q, :], ans[0:q, :])

    # --- schedule ---
    for i in range(4):
        load_transpose(i)
    qproj(0)
    for i in range(4, NCH):
        load_transpose(i)
    # Attention pipeline on 112-query chunks; score transposes run two
    # steps behind logits (softmax latency cover) and SV four steps
    # behind (DVE copy cover + PE filler for the drain's last exps), so
    # the PE stream never drains or resets its clock ramp.
    for k in range(NQC):
        if k == 4:
            qproj(1)
        vproj(k)
        logits(k)
        if k >= 2:
            scoreT(k - 2)
        if k >= 4:
            sv_store(k - 4)
    scoreT(NQC - 2)
    sv_store(NQC - 4)
    scoreT(NQC - 1)
    sv_store(NQC - 3)
    sv_store(NQC - 2)
    sv_store(NQC - 1)

    stack.close()


def _build():
    if "nc" in _cache:
        return _cache["nc"]
    nc = bacc.Bacc("TRN2", target_bir_lowering=False, debug=False, num_devices=B)
    xd = nc.dram_tensor("x", [T, D], bf16, kind="ExternalInput")
    Gd = nc.dram_tensor("G", [D, D], bf16, kind="ExternalInput")
    Vd = nc.dram_tensor("Vw", [D, D], bf16, kind="ExternalInput")
    outd = nc.dram_tensor("out", [T, D], bf16, kind="ExternalOutput")
    with tile.TileContext(nc) as tc:
        _emit(tc, nc, xd, Gd, Vd, outd)
    nc.compile()
    _cache["nc"] = nc
    return nc


def make_in_maps(inputs, M, C, V):
    x = np.asarray(inputs, dtype=np.float32)
    M = np.asarray(M, dtype=np.float32)
    C = np.asarray(C, dtype=np.float32)
    V = np.asarray(V, dtype=np.float32)
    assert x.shape == (B, T, D), x.shape
    G = (M.astype(np.float64) @ C.astype(np.float64).T).astype(ml_dtypes.bfloat16)
    Gb = np.ascontiguousarray(G)
    Vb = np.ascontiguousarray(V.astype(ml_dtypes.bfloat16))
    xb = np.ascontiguousarray(x.astype(ml_dtypes.bfloat16))
    return [{"x": xb[b], "G": Gb, "Vw": Vb} for b in range(B)]


def kernel(inputs, M, C, V):
    nc = _build()
    in_maps = make_in_maps(inputs, M, C, V)
    res = run_bass_kernel_spmd(nc, in_maps, core_ids=list(range(B)))
    x = np.asarray(inputs, dtype=np.float32)
    ans = np.stack(
        [np.asarray(res.results[b]["out"]).astype(np.float32) for b in range(B)],
        axis=0,
    )
    return np.concatenate([x, ans], axis=-1)


# revision 14
# speedup vs baseline: 1.0094x; 1.0094x over previous
"""Trainium2 Bass kernel for nn_AttentionCell (sparse local attention, W=16).

Contract: kernel(**inputs) takes the FULL inputs
    inputs: [8, 1024, 512] f32, M/C/V: [512, 512] f32
and returns the FULL output [8, 1024, 1024] f32
    out = concat([inputs, local_attention(inputs)], axis=-1)

Sharding: data-parallel over batch - one batch element per NeuronCore (8
cores). Host-side prep: M/C are fused into G = M @ C.T so that
    logits = (x @ M) @ (x @ C).T = (x @ G) @ x.T
(no K projection on device); x, G, V are cast to bf16 on the host; the
device computes ONLY the attention half (bf16 out) and the host
concatenates [x_f32, answer_f32].  Device HBM traffic is 1MB x in +
1MB G/V in + 1MB answer out (vs 8MB for the all-f32 full-output kernel).

Per-core device algorithm (x: [1024, 512] bf16), all matmuls bf16
(1 cyc/row on PE at any free size, 2x faster transposes vs f32):
  1. xT stored zero-padded by PAD=16 cols left and 96 right so any
     128-wide window is a contiguous slice (col j = x row j-16),
     built from eight 128-row chunk transposes.
  2. Q'T = G.T @ xT (two 512-wide spans).
  3. Attention runs on 112-query chunks (9x112 + 1x16): queries
     [112k, 112k+111] attend keys [112k-16, 112k+111] - exactly 128
     rows, so V' for chunk k is ONE 128-row window of x @ Vw computed
     straight off xT (Vn[p, k] = V' row 112k-16+p), and S @ V is ONE
     K=128 matmul with no partition-misaligned tail.
  4. Banded logits [112, 128] (4 accumulating matmuls); softmax with an
     additive -1e9 band mask built on-device via affine_select (valid
     w in [q+1, q+16]; out-of-sequence keys are zero columns of xT so
     their logits are exactly 0, matching the reference zero-padding);
     scores transposed on PE, normalization folded into the PSUM->SBUF
     scale copy, output cast to bf16.

DMA plan: the two HWDGE rings are used in parallel at startup
(sync: x chunks 0-3 in two paired loads + answer stores; scalar:
G halves, x chunks 4-7, V) so sequencer config time (~0.6us per DMA)
never serializes the x stream behind the weights.  PE warm-up junk
matmuls (no data deps) open the clock gate during the load latency.
"""

import os
import sys

import numpy as np

for _p in ("/opt/trn_rl_repo", "/opt/pypackages"):
    if os.path.isdir(_p) and _p not in sys.path:
        sys.path.append(_p)

import ml_dtypes

import concourse.bacc as bacc
import concourse.tile as tile
from concourse import mybir
from concourse.bass_utils import run_bass_kernel_spmd

f32 = mybir.dt.float32
bf16 = mybir.dt.bfloat16

B = 8
T = 1024
D = 512
PAD = 16           # left halo: 15 in-window keys + 1 masked (alignment)
QCH = 112          # attention query-chunk size (key span = QCH+16 = 128)
NQC = 10           # 9 chunks of 112 + final chunk of 16
RPAD = 96          # right zero pad so chunk 9's 128-wide reads stay in range
XTW = PAD + T + RPAD
NCH = T // 128     # 128-row transpose/load chunks
NDC = D // 128     # feature chunks
MASKVAL = -1.0e9

_cache: dict = {}


def _ts(i, n=128):
    return slice(i * n, (i + 1) * n)


def _qn(k):
    return QCH if k < NQC - 1 else T - QCH * (NQC - 1)


def _emit(tc, nc, xd, Gd, Vd, outd):
    AF = mybir.ActivationFunctionType
    OP = mybir.AluOpType
    from contextlib import ExitStack

    stack = ExitStack()
    constp = stack.enter_context(tc.tile_pool(name="const", bufs=1))
    bigp = stack.enter_context(tc.tile_pool(name="big", bufs=1))
    smp = stack.enter_context(tc.tile_pool(name="sm", bufs=4))
    pTp = stack.enter_context(tc.tile_pool(name="pT", bufs=2, space="PSUM"))
    pQVp = stack.enter_context(tc.tile_pool(name="pQV", bufs=3, space="PSUM"))
    pLp = stack.enter_context(tc.tile_pool(name="pL", bufs=3, space="PSUM"))

    # --- constants (generated on device: no DMA); the warm-up zero tile
    # is memset FIRST so the PE can start immediately ---
    zt = constp.tile([128, 128], bf16)
    nc.vector.memset(zt[:], 0.0)
    identity = constp.tile([128, 128], bf16)
    nc.gpsimd.memset(identity[:], 0.0)
    nc.gpsimd.affine_select(
        out=identity[:], in_=identity[:], compare_op=OP.not_equal,
        fill=1.0, base=0, pattern=[[-1, 128]], channel_multiplier=1,
    )
    # band[q, w] = 0 where q+1 <= w <= q+16 else -1e9
    band = constp.tile([128, 128], f32)
    nc.gpsimd.memset(band[:], 0.0)
    nc.gpsimd.affine_select(
        out=band[:], in_=band[:], compare_op=OP.is_ge,
        fill=MASKVAL, base=-1, pattern=[[1, 128]], channel_multiplier=-1,
    )
    nc.gpsimd.affine_select(
        out=band[:], in_=band[:], compare_op=OP.is_ge,
        fill=MASKVAL, base=PAD, pattern=[[-1, 128]], channel_multiplier=1,
    )
    # --- weights ---
    Gw = constp.tile([128, NDC, D], bf16)
    Vws = constp.tile([128, NDC, D], bf16)

    # --- persistent activations ---
    xin = bigp.tile([128, NCH, D], bf16)     # x chunks, natural layout
    xTp = bigp.tile([128, NDC, XTW], bf16)   # x.T, zero-padded both sides
    nc.vector.memset(xTp[:, :, 0:PAD], 0.0)
    nc.vector.memset(xTp[:, :, PAD + T : XTW], 0.0)
    QT = bigp.tile([128, NDC, T], bf16)      # (x @ G).T
    Vn = bigp.tile([128, NQC, D], bf16)      # Vn[p, k] = (x@Vw) row 112k-16+p

    # --- loads: both HWDGE rings in parallel ---
    xdr = xd.rearrange("(c p) d -> p c d", p=128)
    Gdr = Gd.rearrange("(c p) n -> p c n", p=128)
    nc.sync.dma_start(xin[:, 0:2, :], xdr[:, 0:2, :])
    nc.sync.dma_start(xin[:, 2:4, :], xdr[:, 2:4, :])
    nc.scalar.dma_start(Gw[:, :, 0:256], Gdr[:, :, 0:256])
    nc.scalar.dma_start(Gw[:, :, 256:512], Gdr[:, :, 256:512])
    nc.scalar.dma_start(xin[:, 4:8, :], xdr[:, 4:8, :])
    nc.scalar.dma_start(Vws[:], Vd.rearrange("(c p) n -> p c n", p=128))

    # --- PE warm-up: junk matmuls (no data deps) open the HAM clock-gate
    # (0.65 -> 2.4 GHz) while the first x chunks load; sized to end right
    # when the first x load's semaphore fires (~2.9us after main) since
    # any PE idle gap resets the clock ramp ---
    pwarm = pLp.tile([128, 128], f32, name="pwarm", tag="pl")
    NWARM = 26
    for w in range(NWARM):
        nc.tensor.matmul(
            pwarm[:], zt[:], zt[:], start=(w == 0), stop=(w == NWARM - 1)
        )

    # --- per-128-row-chunk transpose ---
    def load_transpose(i):
        pst = pTp.tile([128, NDC, 128], bf16, name=f"pt{i}", tag="pt")
        for dc in range(NDC):
            nc.tensor.transpose(pst[:, dc, :], xin[:, i, _ts(dc)], identity[:])
        nc.vector.tensor_copy(xTp[:, :, PAD + 128 * i : PAD + 128 * (i + 1)], pst[:])

    # --- Q' projection for one 512-wide t-span ---
    def qproj(s):
        for m in range(NDC):
            pq = pQVp.tile([128, 512], f32, name=f"pq{s}_{m}", tag="pq")
            for dc in range(NDC):
                nc.tensor.matmul(
                    pq[:],
                    Gw[:, dc, _ts(m)],
                    xTp[:, dc, PAD + 512 * s : PAD + 512 * (s + 1)],
                    start=(dc == 0),
                    stop=(dc == NDC - 1),
                )
            nc.scalar.copy(QT[:, m, _ts(s, 512)], pq[:])

    # --- V' window projection for one 112-query chunk: rows 112k-16.. ---
    def vproj(k):
        pv = pQVp.tile([128, 512], f32, name=f"pv{k}", tag="pq")
        for dc in range(NDC):
            nc.tensor.matmul(
                pv[:],
                xTp[:, dc, QCH * k : QCH * k + 128],
                Vws[:, dc, :],
                start=(dc == 0),
                stop=(dc == NDC - 1),
            )
        # k=7,9 go to DVE so the ACT queue stays clear for the final exps
        if k % 2 == 0 or k >= 7:
            nc.vector.tensor_copy(Vn[:, k, :], pv[:])
        else:
            nc.scalar.copy(Vn[:, k, :], pv[:])

    # --- banded logits for one 112-query chunk ---
    pltiles = {}

    def logits(k):
        q = _qn(k)
        pl = pLp.tile([128, 128], f32, name=f"pl{k}", tag="pl")
        for dc in range(NDC):
            nc.tensor.matmul(
                pl[0:q, :],
                QT[:, dc, QCH * k : QCH * k + q],
                xTp[:, dc, QCH * k : QCH * k + 128],
                start=(dc == 0),
                stop=(dc == NDC - 1),
            )
        pltiles[k] = pl

    # --- softmax + score transpose (PE part emitted separately from SV
    # so the DVE psum->sbuf copy of the scores never stalls the PE) ---
    sttiles = {}
    rctiles = {}

    def scoreT(k):
        q = _qn(k)
        pl = pltiles.pop(k)
        Lm = smp.tile([128, 128], f32, name=f"lm{k}", tag="lm")
        nc.vector.tensor_add(Lm[0:q, :], pl[0:q, :], band[0:q, :])
        negm = smp.tile([128, 1], f32, name=f"nm{k}", tag="nm")
        nc.vector.reduce_max(
            negm[0:q, :], Lm[0:q, :], axis=mybir.AxisListType.X, negate=True
        )
        P = smp.tile([128, 128], bf16, name=f"pp{k}", tag="pp")
        rowsum = smp.tile([128, 1], f32, name=f"rs{k}", tag="rs")
        nc.scalar.activation(
            P[0:q, :], Lm[0:q, :], AF.Exp, bias=negm[0:q, :], accum_out=rowsum[0:q, :]
        )
        recip = smp.tile([128, 1], f32, name=f"rc{k}", tag="rc")
        nc.vector.reciprocal(recip[0:q, :], rowsum[0:q, :])
        rctiles[k] = recip
        pst = pTp.tile([128, QCH], bf16, name=f"ps{k}", tag="pt")
        nc.tensor.transpose(pst[:, 0:q], P[0:q, :], identity[0:q, 0:q])
        st = smp.tile([128, QCH], bf16, name=f"st{k}", tag="st")
        nc.vector.tensor_copy(st[:, 0:q], pst[:, 0:q])
        sttiles[k] = st

    def sv_store(k):
        q = _qn(k)
        st = sttiles.pop(k)
        recip = rctiles.pop(k)
        pa = pQVp.tile([128, 512], f32, name=f"pa{k}", tag="pq")
        nc.tensor.matmul(pa[0:q, :], st[:, 0:q], Vn[:, k, :], start=True, stop=True)
        ans = smp.tile([128, 512], bf16, name=f"ans{k}", tag="ans")
        if k % 2 == 0:
            nc.scalar.mul(ans[0:q, :], pa[0:q, :], recip[0:q, :])
        else:
            nc.vector.tensor_scalar_mul(ans[0:q, :], pa[0:q, :], recip[0:q, :])
        # the last chunk's store goes on the otherwise-idle scalar ring so
        # its DMA config runs in parallel with the previous store's
        eng = nc.scalar if k == NQC - 1 else nc.sync
        eng.dma_start(outd[QCH * k : QCH * k + q, :], ans[0:q, :])

    # --- schedule ---
    for i in range(4):
        load_transpose(i)
    qproj(0)
    for i in range(4, NCH):
        load_transpose(i)
    # Attention pipeline on 112-query chunks; score transposes run two
    # steps behind logits (softmax latency cover) and SV four steps
    # behind (DVE copy cover + PE filler for the drain's last exps), so
    # the PE stream never drains or resets its clock ramp.
    for k in range(NQC):
        if k == 4:
            qproj(1)
        vproj(k)
        logits(k)
        if k >= 2:
            scoreT(k - 2)
        if k >= 4:
            sv_store(k - 4)
    scoreT(NQC - 2)
    sv_store(NQC - 4)
    scoreT(NQC - 1)
    sv_store(NQC - 3)
    sv_store(NQC - 2)
    sv_store(NQC - 1)

    stack.close()


def _build():
    if "nc" in _cache:
        return _cache["nc"]
    nc = bacc.Bacc("TRN2", target_bir_lowering=False, debug=False, num_devices=B)
    xd = nc.dram_tensor("x", [T, D], bf16, kind="ExternalInput")
    Gd = nc.dram_tensor("G", [D, D], bf16, kind="ExternalInput")
    Vd = nc.dram_tensor("Vw", [D, D], bf16, kind="ExternalInput")
    outd = nc.dram_tensor("out", [T, D], bf16, kind="ExternalOutput")
    with tile.TileContext(nc) as tc:
        _emit(tc, nc, xd, Gd, Vd, outd)
    nc.compile()
    _cache["nc"] = nc
    return nc


def make_in_maps(inputs, M, C, V):
    x = np.asarray(inputs, dtype=np.float32)
    M = np.asarray(M, dtype=np.float32)
    C = np.asarray(C, dtype=np.float32)
    V = np.asarray(V, dtype=np.float32)
    assert x.shape == (B, T, D), x.shape
    G = (M.astype(np.float64) @ C.astype(np.float64).T).astype(ml_dtypes.bfloat16)
    Gb = np.ascontiguousarray(G)
    Vb = np.ascontiguousarray(V.astype(ml_dtypes.bfloat16))
    xb = np.ascontiguousarray(x.astype(ml_dtypes.bfloat16))
    return [{"x": xb[b], "G": Gb, "Vw": Vb} for b in range(B)]


def kernel(inputs, M, C, V):
    nc = _build()
    in_maps = make_in_maps(inputs, M, C, V)
    res = run_bass_kernel_spmd(nc, in_maps, core_ids=list(range(B)))
    x = np.asarray(inputs, dtype=np.float32)
    ans = np.stack(
        [np.asarray(res.results[b]["out"]).astype(np.float32) for b in range(B)],
        axis=0,
    )
    return np.concatenate([x, ans], axis=-1)
